# revision 24
# baseline (speedup 1.0000x reference)
"""DC_CE_Marginal_loss for Trainium2 — 8-core data-parallel Bass kernel (v2).

Shards the [B,C,D,H,W] volume along D across 8 NeuronCores, two launches:

  Launch A (counts): per-core fp8 one-hot target (exact in fp8-e4m3) is
      streamed once; per-(b,c) voxel counts come from free-dim reductions
      split across DVE / ACT. Host psums the 8x[128,16] partials into
      global counts and derives the per-sample present-class pattern +
      CE padding.

  Launch B (main, compiled per present-pattern): all-bf16 pipeline.
      Per sample: bg-merge only over the absent channels (weights are
      exactly 1), one wide exp over the present channels (present bias
      is 0 so no mask add is needed), pairwise-tree softmax denominator,
      fast reciprocal, then per-channel tensor_tensor_reduce ops that
      produce q = e*r and tq = t*q while accumulating seg_vol / intersect
      into per-(b,c) columns. q/tq overwrite the dead x/t planes in SBUF.
      The CE dot product sum(t*m) is recovered analytically:
      m_label = ln(q_label) + ln(S), and q_label = sum_c tq_c (an exact
      one-hot select), so two ACT Ln+accum passes replace the whole t*m
      multiply/accumulate pipeline.

Host sums the per-core partial columns and finishes the loss.
"""
import numpy as np
import ml_dtypes

B, C, D, H, W = 2, 8, 64, 160, 160
NCORES = 8
DS = D // NCORES            # depth slices per core
PLANE = DS * H * W          # voxels per (b,c) plane per core = 204800
P = 128
FREE = PLANE // P           # 1600
NVOX = B * D * H * W

# launch B accumulator columns.
# SEGB/INTB columns hold per-(b,c) values at PARTITION c (from the matmul
# reduction), so the host reads them per-partition instead of summing.
SEGB = 0           # B cols: seg_vol, value for channel c at partition c
INTB = 2           # B cols: intersect, value for channel c at partition c
U1 = 4             # 2: sum ln(q_label) per sample (summed over partitions)
U2 = 6             # 2: sum ln(S) per sample (only used when pad>0)
LSE = 8            # 2: sum ln(S + pad) per sample
NACC = 10

_CACHE = {}


def _build_a():
    import concourse.bacc as bacc
    import concourse.tile as tile
    from concourse import mybir

    FA = mybir.ActivationFunctionType
    AL = mybir.AluOpType
    f32, bf16 = mybir.dt.float32, mybir.dt.bfloat16

    nc = bacc.Bacc("TRN2", num_devices=NCORES, name="loss_counts_v3")
    # labels per sample, bf16 (values 0..C-1 exact), partition-major
    lab = nc.dram_tensor("lab", [P, B, FREE], bf16, kind="ExternalInput")
    # counts for (b, c) of sample 1 live at partition c of column b;
    # sample-0 counts are per-partition partials in cnt2 columns
    out = nc.dram_tensor("cnt", [P, B], f32, kind="ExternalOutput")
    out2 = nc.dram_tensor("cnt2", [P, B * C], f32, kind="ExternalOutput")

    with tile.TileContext(nc) as tc:
        with (
            tc.tile_pool(name="sb", bufs=1) as sb,
            tc.psum_pool(name="ps", bufs=2) as psp,
        ):
            lab_sb = sb.tile([P, B, FREE], bf16)
            mk = sb.tile([P, B * C, FREE], bf16)
            cnt = sb.tile([P, B], f32)
            nc.vector.memset(cnt[:], 0.0)
            wsl = sb.tile([P, 2 * C - 1], bf16)
            nc.gpsimd.memset(wsl[:], 0.0)
            nc.gpsimd.memset(wsl[:, C - 1 : C], 1.0)
            CH = [(0, 400), (400, 800), (800, 1200), (1200, 1600)]
            for b in range(B):
                nc.sync.dma_start(lab_sb[:, b, :], lab[:, b, :])
            junk_a = sb.tile([P, FREE], f32)
            # cnt2: per-(b,c) counts as ordinary columns (summed over
            # partitions host-side) for the ACT/DVE-reduced planes
            cnt2 = sb.tile([P, B * C], f32)
            nc.vector.memset(cnt2[:], 0.0)
            # interleave (b0, b1) compares so the ACT (sample 0) and PE
            # (sample 1) reductions both chase the DVE compares plane by
            # plane instead of waiting for a whole sample
            ps = psp.tile([C, 400], f32, tag="ps")
            for c in range(C):
                for b in range(B):
                    nc.vector.tensor_scalar(
                        mk[:, b * C + c, :], lab_sb[:, b, :],
                        float(c), None, AL.is_equal)
                if c < 6:
                    nc.scalar.activation(
                        out=junk_a[:], in_=mk[:, c, :], func=FA.Copy,
                        accum_out=cnt2[:, c : c + 1])
                for j in range(4):
                    lo, hi = CH[j]
                    nc.tensor.matmul(
                        ps[:, 0 : hi - lo],
                        wsl[:, C - 1 - c : 2 * C - 1 - c],
                        mk[:, C + c, lo:hi],
                        start=(c == 0 and j == 0),
                        stop=(c == C - 1 and j == 3))
            # b0 channels 6,7 reduced on DVE right after their compares
            nc.vector.tensor_reduce(
                out=cnt2[:, 6:8], in_=mk[:, 6:8, :],
                axis=mybir.AxisListType.X, op=AL.add)
            nc.vector.tensor_reduce(
                out=cnt[0:C, 1:2], in_=ps[:],
                axis=mybir.AxisListType.X, op=AL.add)
            nc.sync.dma_start(out[:], cnt[:])
            nc.sync.dma_start(out2[:], cnt2[:])
    nc.compile()
    return nc


def _tree(nc, s4, s2, planes, out_ap=None):
    """Pairwise-add a list of [P, F] APs using slices of the scratch tiles
    s4 ([P,4,F]) / s2 ([P,2,F]) for intermediates. The final add writes
    out_ap if given. Returns the final AP. len(planes) in [2, 8]."""
    from concourse import mybir
    AL = mybir.AluOpType
    scratch = [s4, s2, None]
    cur = list(planes)
    li = 0
    while len(cur) > 1:
        nxt = []
        k = 0
        for i in range(0, len(cur) - 1, 2):
            final = len(cur) == 2
            if final and out_ap is not None:
                dst = out_ap
            elif final and out_ap is None:
                # pick a slot that is never an input at this level
                dst = (s2 if li < 2 else s4)[:, 0, :]
            else:
                dst = scratch[li][:, k, :]
                k += 1
            nc.vector.tensor_tensor(out=dst, in0=cur[i], in1=cur[i + 1],
                                    op=AL.add)
            nxt.append(dst)
        if len(cur) % 2:
            nxt.append(cur[-1])
        cur = nxt
        li += 1
    return cur[0]


def _build_b(pattern):
    """pattern: tuple per sample of present-channel tuples."""
    import concourse.bacc as bacc
    import concourse.tile as tile
    from concourse import mybir

    FA = mybir.ActivationFunctionType
    AL = mybir.AluOpType
    f32, bf16 = mybir.dt.float32, mybir.dt.bfloat16

    pres = [list(p) for p in pattern]
    n = [len(p) for p in pres]
    L = max(n)
    pad = [float(L - nn) for nn in n]
    absent = [[c for c in range(C) if c not in p] for p in pres]

    nc = bacc.Bacc("TRN2", num_devices=NCORES, name="loss_main_v3")
    # partition-major logits + per-sample labels (bf16 class indices)
    x = nc.dram_tensor("x", [P, B * C, FREE], bf16, kind="ExternalInput")
    lab = nc.dram_tensor("lab", [P, B, FREE], bf16, kind="ExternalInput")
    out = nc.dram_tensor("out", [P, NACC], f32, kind="ExternalOutput")

    with tile.TileContext(nc) as tc:
        with (
            tc.tile_pool(name="persist", bufs=1) as persist,
            tc.tile_pool(name="ework", bufs=2) as ework,
            tc.tile_pool(name="small", bufs=2) as small,
            tc.psum_pool(name="ps", bufs=2) as psp,
        ):
            x_sb = persist.tile([P, B * C, FREE], bf16)
            lab_sb = persist.tile([P, B, FREE], bf16)
            # per-(b,c) one-hot mask planes (only present channels),
            # packed per sample; overwritten by tq later
            tslot = {}
            for b in range(B):
                for c in pres[b]:
                    tslot[(b, c)] = len(tslot)
            mk = persist.tile([P, len(tslot), FREE], bf16)
            accs = persist.tile([P, NACC], f32)
            s4 = persist.tile([P, 4, FREE], bf16)
            s2 = persist.tile([P, 2, FREE], bf16)
            junk = persist.tile([P, FREE], bf16)
            psj = persist.tile([P, 400], bf16)
            nc.vector.memset(accs[:], 0.0)
            wsl = persist.tile([P, 2 * C - 1], bf16)
            nc.gpsimd.memset(wsl[:], 0.0)
            nc.gpsimd.memset(wsl[:, C - 1 : C], 1.0)
            CH = [(0, 400), (400, 800), (800, 1200), (1200, 1600)]

            def class_sums(planes, acc_col):
                """acc_col[i] (partition i) = sum over planes[i], via PE."""
                ps = psp.tile([C, 400], f32, tag="ps")
                items = [(c, j) for c in range(len(planes))
                         for j in range(len(CH))]
                for idx, (c, j) in enumerate(items):
                    lo, hi = CH[j]
                    nc.tensor.matmul(
                        ps[:, 0 : hi - lo],
                        wsl[:, C - 1 - c : 2 * C - 1 - c],
                        planes[c][:, lo:hi],
                        start=(idx == 0), stop=(idx == len(items) - 1))
                nc.scalar.activation(
                    out=psj[0:C, :], in_=ps[:], func=FA.Copy,
                    accum_out=acc_col)

            # DMA order: labels first (mask compares start immediately),
            # then per sample absent-x runs, present-x runs
            for b in range(B):
                nc.sync.dma_start(lab_sb[:, b, :], lab[:, b, :])

            def runs_of(ixs):
                rr = []
                for i in sorted(ixs):
                    if rr and rr[-1][1] == i:
                        rr[-1][1] = i + 1
                    else:
                        rr.append([i, i + 1])
                return rr

            for b in range(B):
                gx = (runs_of([b * C + c for c in absent[b]]) +
                      runs_of([b * C + c for c in pres[b]]))
                for lo, hi in gx:
                    nc.sync.dma_start(x_sb[:, lo:hi, :], x[:, lo:hi, :])

            # one-hot masks from labels (DVE; GpSimd tensor_scalar is
            # ~25x slower than DVE for this)
            for b in range(B):
                for c in pres[b]:
                    nc.vector.tensor_scalar(
                        mk[:, tslot[(b, c)], :], lab_sb[:, b, :],
                        float(c), None, AL.is_equal)

            for b in range(B):
                xb = x_sb[:, b * C : (b + 1) * C, :]
                # ---- bg merge: x0 += sum(absent x) (weights exactly 1)
                if len(absent[b]) == 1:
                    nc.vector.tensor_tensor(
                        out=xb[:, 0, :], in0=xb[:, 0, :],
                        in1=xb[:, absent[b][0], :], op=AL.add)
                elif absent[b]:
                    bg = _tree(nc, s4, s2, [xb[:, c, :] for c in absent[b]])
                    nc.vector.tensor_tensor(
                        out=xb[:, 0, :], in0=xb[:, 0, :], in1=bg, op=AL.add)

                # ---- e = exp(x) over present channels (contiguous runs)
                e = ework.tile([P, C, FREE], bf16, tag="e")
                runs = []
                for c in pres[b]:
                    if runs and runs[-1][1] == c:
                        runs[-1][1] = c + 1
                    else:
                        runs.append([c, c + 1])
                for lo, hi in runs:
                    nc.scalar.activation(
                        out=e[:, lo:hi, :], in_=xb[:, lo:hi, :], func=FA.Exp)

                # ---- S = sum_present e (f32); r = 1/S on DVE (ACT r
                #      would put its Exp<->Ln table flips on the critical
                #      path); Ln(S) accums feed the CE terms
                S = small.tile([P, FREE], f32, tag="S")
                _tree(nc, s4, s2, [e[:, c, :] for c in pres[b]], out_ap=S[:])
                rf = small.tile([P, FREE], f32, tag="rf")
                nc.vector.reciprocal_approx_fast(rf[:], S[:])
                r = small.tile([P, FREE], bf16, tag="r")
                nc.vector.tensor_scalar(r[:], rf[:], 1.0, None, AL.mult)
                if pad[b] > 0:
                    nc.scalar.activation(
                        out=junk[:], in_=S[:], func=FA.Ln,
                        accum_out=accs[:, U2 + b : U2 + b + 1])
                    padb = small.tile([P, 1], f32, tag="pad")
                    nc.vector.memset(padb[:], pad[b])
                    nc.scalar.activation(
                        out=junk[:], in_=S[:], func=FA.Ln, bias=padb[:],
                        accum_out=accs[:, LSE + b : LSE + b + 1])
                else:
                    nc.scalar.activation(
                        out=junk[:], in_=S[:], func=FA.Ln,
                        accum_out=accs[:, LSE + b : LSE + b + 1])

                # ---- q_c = e_c * r, overwriting x planes; seg via PE
                for c in pres[b]:
                    nc.vector.tensor_tensor(
                        out=xb[:, c, :], in0=e[:, c, :], in1=r[:],
                        op=AL.mult)
                class_sums([xb[:, c, :] for c in pres[b]],
                           accs[0:C, SEGB + b : SEGB + b + 1])

                # ---- tq_c = mask_c * q_c, overwriting mask planes;
                #      intersect via PE (GpSimd SBUF traffic poisons DVE
                #      throughput, so everything elementwise stays on DVE)
                for c in pres[b]:
                    sl = tslot[(b, c)]
                    nc.vector.tensor_tensor(
                        out=mk[:, sl, :], in0=mk[:, sl, :],
                        in1=xb[:, c, :], op=AL.mult)
                class_sums([mk[:, tslot[(b, c)], :] for c in pres[b]],
                           accs[0:C, INTB + b : INTB + b + 1])

                # ---- g_q = sum_c tq_c = q_label (exact one-hot select)
                gq = small.tile([P, FREE], bf16, tag="gq")
                _tree(nc, s4, s2,
                      [mk[:, tslot[(b, c)], :] for c in pres[b]],
                      out_ap=gq[:])
                nc.scalar.activation(
                    out=junk[:], in_=gq[:], func=FA.Ln,
                    accum_out=accs[:, U1 + b : U1 + b + 1])

            nc.sync.dma_start(out[:], accs[:])
    nc.compile()
    return nc


def _get(name, builder, *args):
    if name not in _CACHE:
        _CACHE[name] = builder(*args)
    return _CACHE[name]


def _shard_inputs(net_output, target):
    # [B,C,K,P,F] -> per-core partition-major [P, B*C, F] logits plus
    # per-core [P, B, F] label planes (exact bf16 class indices)
    xs = np.asarray(net_output).reshape(B, C, NCORES, P, FREE)
    xpm = np.ascontiguousarray(
        xs.transpose(2, 3, 0, 1, 4).reshape(NCORES, P, B * C, FREE))
    xmaps = [xpm[k].astype(ml_dtypes.bfloat16) for k in range(NCORES)]
    # labels = argmax over one-hot = dot with channel indices (exact)
    ts = np.asarray(target).reshape(B, C, NCORES, P, FREE)
    lab = np.einsum("bckpf,c->bkpf", ts, np.arange(C, dtype=np.float32))
    labpm = np.ascontiguousarray(lab.transpose(1, 2, 0, 3)).astype(
        ml_dtypes.bfloat16)                     # [K, P, B, F]
    labmaps = [labpm[k] for k in range(NCORES)]
    return xmaps, labmaps


def _run(nc, in_maps, out_name):
    import os
    if os.environ.get("K_SIM", "0") == "1":
        import concourse.bass_interp as bass_interp
        sim = bass_interp.MultiCoreSim(nc, NCORES)
        for k in range(NCORES):
            for name, arr in in_maps[k].items():
                sim.cores[k].tensor(name)[:] = arr
        sim.simulate()
        return [{out_name: sim.cores[k].tensor(out_name).copy()}
                for k in range(NCORES)]
    from concourse.bass_utils import run_bass_kernel_spmd
    return run_bass_kernel_spmd(
        nc, in_maps, core_ids=list(range(NCORES))).results


def run_a(labmaps):
    nc = _get("a", _build_a)
    import os
    if os.environ.get("K_SIM", "0") == "1":
        import concourse.bass_interp as bass_interp
        sim = bass_interp.MultiCoreSim(nc, NCORES)
        for k in range(NCORES):
            sim.cores[k].tensor("lab")[:] = labmaps[k]
        sim.simulate()
        results = [{"cnt": sim.cores[k].tensor("cnt").copy(),
                    "cnt2": sim.cores[k].tensor("cnt2").copy()}
                   for k in range(NCORES)]
    else:
        from concourse.bass_utils import run_bass_kernel_spmd
        results = run_bass_kernel_spmd(
            nc, [{"lab": lk} for lk in labmaps],
            core_ids=list(range(NCORES))).results
    cnt_g = np.zeros((B, C), dtype=np.float64)
    for r in results:
        # sample 1 (and any Tensor-reduced sample): per-partition values
        cnt_g[1] += r["cnt"].astype(np.float64)[:C, 1]
        # sample 0: ordinary summed columns
        cnt_g[0] += r["cnt2"].astype(np.float64).sum(axis=0)[:C]
    return cnt_g


def run_b(xmaps, labmaps, pattern):
    nc = _get(("b", pattern), _build_b, pattern)
    in_maps = [{"x": xmaps[k], "lab": labmaps[k]} for k in range(NCORES)]
    results = _run(nc, in_maps, "out")
    acc = np.zeros((P, NACC), dtype=np.float64)
    for r in results:
        acc += r["out"].astype(np.float64)
    return acc


def _finish(cnt_g, acc, present, n):
    pad = n.max() - n
    # SEGB/INTB: per-class values live at partition = position in the
    # present-channel list of that sample
    seg = np.zeros((B, C)); inter = np.zeros((B, C))
    for b in range(B):
        pres = np.where(present[b])[0]
        seg[b, pres] = acc[: len(pres), SEGB + b]
        inter[b, pres] = acc[: len(pres), INTB + b]
    cols = acc.sum(axis=0)
    u1 = cols[U1 : U1 + B]
    lse = cols[LSE : LSE + B]
    u2 = np.where(pad > 0, cols[U2 : U2 + B], lse)
    ce = (lse.sum() - (u1.sum() + u2.sum())) / NVOX
    dice_c = 2.0 * inter / (cnt_g + seg + 1e-5)
    dice_i = 1.0 - (present * dice_c).sum(axis=1) / n
    dc = dice_i.mean()
    return np.asarray(0.5 * ce + 0.5 * dc, dtype=np.float32)


def kernel(net_output, target):
    xmaps, labmaps = _shard_inputs(
        np.asarray(net_output), np.asarray(target))
    cnt_g = run_a(labmaps)
    present = cnt_g > 0.5
    n = present.sum(axis=1).astype(np.float64)
    pattern = tuple(tuple(int(c) for c in np.where(present[b])[0])
                    for b in range(B))
    acc = run_b(xmaps, labmaps, pattern)
    return _finish(cnt_g, acc, present, n)


# revision 25
# speedup vs baseline: 1.0996x; 1.0996x over previous
"""DC_CE_Marginal_loss for Trainium2 — 8-core data-parallel Bass kernel (v2).

Shards the [B,C,D,H,W] volume along D across 8 NeuronCores, two launches:

  Launch A (counts): per-core fp8 one-hot target (exact in fp8-e4m3) is
      streamed once; per-(b,c) voxel counts come from free-dim reductions
      split across DVE / ACT. Host psums the 8x[128,16] partials into
      global counts and derives the per-sample present-class pattern +
      CE padding.

  Launch B (main, compiled per present-pattern): all-bf16 pipeline.
      Per sample: bg-merge only over the absent channels (weights are
      exactly 1), one wide exp over the present channels (present bias
      is 0 so no mask add is needed), pairwise-tree softmax denominator,
      fast reciprocal, then per-channel tensor_tensor_reduce ops that
      produce q = e*r and tq = t*q while accumulating seg_vol / intersect
      into per-(b,c) columns. q/tq overwrite the dead x/t planes in SBUF.
      The CE dot product sum(t*m) is recovered analytically:
      m_label = ln(q_label) + ln(S), and q_label = sum_c tq_c (an exact
      one-hot select), so two ACT Ln+accum passes replace the whole t*m
      multiply/accumulate pipeline.

Host sums the per-core partial columns and finishes the loss.
"""
import numpy as np
import ml_dtypes

B, C, D, H, W = 2, 8, 64, 160, 160
NCORES = 8
DS = D // NCORES            # depth slices per core
PLANE = DS * H * W          # voxels per (b,c) plane per core = 204800
P = 128
FREE = PLANE // P           # 1600
NVOX = B * D * H * W

# launch B accumulator columns.
# SEGB/INTB columns hold per-(b,c) values at PARTITION c (from the matmul
# reduction), so the host reads them per-partition instead of summing.
SEGB = 0           # B cols: seg_vol, value for channel c at partition c
INTB = 2           # B cols: intersect, value for channel c at partition c
U1 = 4             # 2: sum ln(q_label) per sample (summed over partitions)
U2 = 6             # 2: sum ln(S) per sample (only used when pad>0)
LSE = 8            # 2: sum ln(S + pad) per sample
NACC = 10

_CACHE = {}


def _build_a():
    import concourse.bacc as bacc
    import concourse.tile as tile
    from concourse import mybir

    FA = mybir.ActivationFunctionType
    AL = mybir.AluOpType
    f32, bf16 = mybir.dt.float32, mybir.dt.bfloat16

    nc = bacc.Bacc("TRN2", num_devices=NCORES, name="loss_counts_v3")
    # labels per sample, bf16 (values 0..C-1 exact), partition-major
    lab = nc.dram_tensor("lab", [P, B, FREE], bf16, kind="ExternalInput")
    # counts for (b, c) of sample 1 live at partition c of column b;
    # sample-0 counts are per-partition partials in cnt2 columns
    out = nc.dram_tensor("cnt", [P, B], f32, kind="ExternalOutput")
    out2 = nc.dram_tensor("cnt2", [P, B * C], f32, kind="ExternalOutput")

    with tile.TileContext(nc) as tc:
        with (
            tc.tile_pool(name="sb", bufs=1) as sb,
            tc.psum_pool(name="ps", bufs=2) as psp,
        ):
            lab_sb = sb.tile([P, B, FREE], bf16)
            mk = sb.tile([P, B * C, FREE], bf16)
            cnt = sb.tile([P, B], f32)
            nc.vector.memset(cnt[:], 0.0)
            wsl = sb.tile([P, 2 * C - 1], bf16)
            nc.gpsimd.memset(wsl[:], 0.0)
            nc.gpsimd.memset(wsl[:, C - 1 : C], 1.0)
            CH = [(0, 400), (400, 800), (800, 1200), (1200, 1600)]
            nc.sync.dma_start(lab_sb[:], lab[:])
            junk_a = sb.tile([P, FREE], f32)
            # cnt2: per-(b,c) counts as ordinary columns (summed over
            # partitions host-side) for the ACT/DVE-reduced planes
            cnt2 = sb.tile([P, B * C], f32)
            nc.vector.memset(cnt2[:], 0.0)
            # interleave (b0, b1) compares so the ACT (sample 0) and PE
            # (sample 1) reductions both chase the DVE compares plane by
            # plane instead of waiting for a whole sample
            for b in range(B):
                for c in range(C):
                    nc.vector.tensor_scalar(
                        mk[:, b * C + c, :], lab_sb[:, b, :],
                        float(c), None, AL.is_equal)
                if b == 0:
                    for c in range(C):
                        nc.scalar.activation(
                            out=junk_a[:], in_=mk[:, c, :], func=FA.Copy,
                            accum_out=cnt2[:, c : c + 1])
                else:
                    ps = psp.tile([C, 400], f32, tag="ps")
                    items = [(c, j) for c in range(C) for j in range(4)]
                    for idx, (c, j) in enumerate(items):
                        lo, hi = CH[j]
                        nc.tensor.matmul(
                            ps[:, 0 : hi - lo],
                            wsl[:, C - 1 - c : 2 * C - 1 - c],
                            mk[:, b * C + c, lo:hi],
                            start=(idx == 0), stop=(idx == len(items) - 1))
                    nc.vector.tensor_reduce(
                        out=cnt[0:C, b : b + 1], in_=ps[:],
                        axis=mybir.AxisListType.X, op=AL.add)
            nc.sync.dma_start(out[:], cnt[:])
            nc.sync.dma_start(out2[:], cnt2[:])
    nc.compile()
    return nc


def _tree(nc, s4, s2, planes, out_ap=None):
    """Pairwise-add a list of [P, F] APs using slices of the scratch tiles
    s4 ([P,4,F]) / s2 ([P,2,F]) for intermediates. The final add writes
    out_ap if given. Returns the final AP. len(planes) in [2, 8]."""
    from concourse import mybir
    AL = mybir.AluOpType
    scratch = [s4, s2, None]
    cur = list(planes)
    li = 0
    while len(cur) > 1:
        nxt = []
        k = 0
        for i in range(0, len(cur) - 1, 2):
            final = len(cur) == 2
            if final and out_ap is not None:
                dst = out_ap
            elif final and out_ap is None:
                # pick a slot that is never an input at this level
                dst = (s2 if li < 2 else s4)[:, 0, :]
            else:
                dst = scratch[li][:, k, :]
                k += 1
            nc.vector.tensor_tensor(out=dst, in0=cur[i], in1=cur[i + 1],
                                    op=AL.add)
            nxt.append(dst)
        if len(cur) % 2:
            nxt.append(cur[-1])
        cur = nxt
        li += 1
    return cur[0]


def _build_b(pattern):
    """pattern: tuple per sample of present-channel tuples."""
    import concourse.bacc as bacc
    import concourse.tile as tile
    from concourse import mybir

    FA = mybir.ActivationFunctionType
    AL = mybir.AluOpType
    f32, bf16 = mybir.dt.float32, mybir.dt.bfloat16

    pres = [list(p) for p in pattern]
    n = [len(p) for p in pres]
    L = max(n)
    pad = [float(L - nn) for nn in n]
    absent = [[c for c in range(C) if c not in p] for p in pres]

    nc = bacc.Bacc("TRN2", num_devices=NCORES, name="loss_main_v3")
    # partition-major logits + per-sample labels (bf16 class indices)
    x = nc.dram_tensor("x", [P, B * C, FREE], bf16, kind="ExternalInput")
    lab = nc.dram_tensor("lab", [P, B, FREE], bf16, kind="ExternalInput")
    out = nc.dram_tensor("out", [P, NACC], f32, kind="ExternalOutput")

    with tile.TileContext(nc) as tc:
        with (
            tc.tile_pool(name="persist", bufs=1) as persist,
            tc.tile_pool(name="ework", bufs=2) as ework,
            tc.tile_pool(name="small", bufs=2) as small,
            tc.psum_pool(name="ps", bufs=2) as psp,
        ):
            x_sb = persist.tile([P, B * C, FREE], bf16)
            lab_sb = persist.tile([P, B, FREE], bf16)
            # per-(b,c) one-hot mask planes (only present channels),
            # packed per sample; overwritten by tq later
            tslot = {}
            for b in range(B):
                for c in pres[b]:
                    tslot[(b, c)] = len(tslot)
            mk = persist.tile([P, len(tslot), FREE], bf16)
            accs = persist.tile([P, NACC], f32)
            s4 = persist.tile([P, 4, FREE], bf16)
            s2 = persist.tile([P, 2, FREE], bf16)
            junk = persist.tile([P, FREE], bf16)
            psj = persist.tile([P, 400], bf16)
            nc.vector.memset(accs[:], 0.0)
            wsl = persist.tile([P, 2 * C - 1], bf16)
            nc.gpsimd.memset(wsl[:], 0.0)
            nc.gpsimd.memset(wsl[:, C - 1 : C], 1.0)
            CH = [(0, 400), (400, 800), (800, 1200), (1200, 1600)]

            def class_sums(planes, acc_col):
                """acc_col[i] (partition i) = sum over planes[i], via PE."""
                ps = psp.tile([C, 400], f32, tag="ps")
                items = [(c, j) for c in range(len(planes))
                         for j in range(len(CH))]
                for idx, (c, j) in enumerate(items):
                    lo, hi = CH[j]
                    nc.tensor.matmul(
                        ps[:, 0 : hi - lo],
                        wsl[:, C - 1 - c : 2 * C - 1 - c],
                        planes[c][:, lo:hi],
                        start=(idx == 0), stop=(idx == len(items) - 1))
                nc.vector.tensor_reduce(
                    out=acc_col, in_=ps[:],
                    axis=mybir.AxisListType.X, op=AL.add)

            # DMA order: labels first (mask compares start immediately),
            # then per sample absent-x runs, present-x runs
            nc.sync.dma_start(lab_sb[:], lab[:])

            def runs_of(ixs):
                rr = []
                for i in sorted(ixs):
                    if rr and rr[-1][1] == i:
                        rr[-1][1] = i + 1
                    else:
                        rr.append([i, i + 1])
                return rr

            for b in range(B):
                gx = (runs_of([b * C + c for c in absent[b]]) +
                      runs_of([b * C + c for c in pres[b]]))
                for lo, hi in gx:
                    nc.sync.dma_start(x_sb[:, lo:hi, :], x[:, lo:hi, :])

            # one-hot masks from labels (DVE; GpSimd tensor_scalar is
            # ~25x slower than DVE for this)
            for b in range(B):
                for c in pres[b]:
                    nc.vector.tensor_scalar(
                        mk[:, tslot[(b, c)], :], lab_sb[:, b, :],
                        float(c), None, AL.is_equal)

            for b in range(B):
                xb = x_sb[:, b * C : (b + 1) * C, :]
                # ---- bg merge: x0 += sum(absent x) (weights exactly 1)
                if len(absent[b]) == 1:
                    nc.vector.tensor_tensor(
                        out=xb[:, 0, :], in0=xb[:, 0, :],
                        in1=xb[:, absent[b][0], :], op=AL.add)
                elif absent[b]:
                    bg = _tree(nc, s4, s2, [xb[:, c, :] for c in absent[b]])
                    nc.vector.tensor_tensor(
                        out=xb[:, 0, :], in0=xb[:, 0, :], in1=bg, op=AL.add)

                # ---- e = exp(x) over present channels (contiguous runs)
                e = ework.tile([P, C, FREE], bf16, tag="e")
                runs = []
                for c in pres[b]:
                    if runs and runs[-1][1] == c:
                        runs[-1][1] = c + 1
                    else:
                        runs.append([c, c + 1])
                for lo, hi in runs:
                    nc.scalar.activation(
                        out=e[:, lo:hi, :], in_=xb[:, lo:hi, :], func=FA.Exp)

                # ---- S = sum_present e (f32); r = 1/S on DVE (ACT r
                #      would put its Exp<->Ln table flips on the critical
                #      path); Ln(S) accums feed the CE terms
                S = small.tile([P, FREE], f32, tag="S")
                _tree(nc, s4, s2, [e[:, c, :] for c in pres[b]], out_ap=S[:])
                rf = small.tile([P, FREE], f32, tag="rf")
                nc.vector.reciprocal_approx_fast(rf[:], S[:])
                r = small.tile([P, FREE], bf16, tag="r")
                nc.vector.tensor_scalar(r[:], rf[:], 1.0, None, AL.mult)
                if pad[b] > 0:
                    nc.scalar.activation(
                        out=junk[:], in_=S[:], func=FA.Ln,
                        accum_out=accs[:, U2 + b : U2 + b + 1])
                    padb = small.tile([P, 1], f32, tag="pad")
                    nc.vector.memset(padb[:], pad[b])
                    nc.scalar.activation(
                        out=junk[:], in_=S[:], func=FA.Ln, bias=padb[:],
                        accum_out=accs[:, LSE + b : LSE + b + 1])
                else:
                    nc.scalar.activation(
                        out=junk[:], in_=S[:], func=FA.Ln,
                        accum_out=accs[:, LSE + b : LSE + b + 1])

                # ---- q_c = e_c * r, overwriting x planes; seg via PE
                for c in pres[b]:
                    nc.vector.tensor_tensor(
                        out=xb[:, c, :], in0=e[:, c, :], in1=r[:],
                        op=AL.mult)
                class_sums([xb[:, c, :] for c in pres[b]],
                           accs[0:C, SEGB + b : SEGB + b + 1])

                # ---- tq_c = mask_c * q_c, overwriting mask planes;
                #      intersect via PE (GpSimd SBUF traffic poisons DVE
                #      throughput, so everything elementwise stays on DVE)
                for c in pres[b]:
                    sl = tslot[(b, c)]
                    nc.vector.tensor_tensor(
                        out=mk[:, sl, :], in0=mk[:, sl, :],
                        in1=xb[:, c, :], op=AL.mult)
                class_sums([mk[:, tslot[(b, c)], :] for c in pres[b]],
                           accs[0:C, INTB + b : INTB + b + 1])

                # ---- g_q = sum_c tq_c = q_label (exact one-hot select)
                gq = small.tile([P, FREE], bf16, tag="gq")
                _tree(nc, s4, s2,
                      [mk[:, tslot[(b, c)], :] for c in pres[b]],
                      out_ap=gq[:])
                nc.scalar.activation(
                    out=junk[:], in_=gq[:], func=FA.Ln,
                    accum_out=accs[:, U1 + b : U1 + b + 1])

            nc.sync.dma_start(out[:], accs[:])
    nc.compile()
    return nc


def _get(name, builder, *args):
    if name not in _CACHE:
        _CACHE[name] = builder(*args)
    return _CACHE[name]


def _shard_inputs(net_output, target):
    # [B,C,K,P,F] -> per-core partition-major [P, B*C, F] logits plus
    # per-core [P, B, F] label planes (exact bf16 class indices)
    xs = np.asarray(net_output).reshape(B, C, NCORES, P, FREE)
    xpm = np.ascontiguousarray(
        xs.transpose(2, 3, 0, 1, 4).reshape(NCORES, P, B * C, FREE))
    xmaps = [xpm[k].astype(ml_dtypes.bfloat16) for k in range(NCORES)]
    # labels = argmax over one-hot = dot with channel indices (exact)
    ts = np.asarray(target).reshape(B, C, NCORES, P, FREE)
    lab = np.einsum("bckpf,c->bkpf", ts, np.arange(C, dtype=np.float32))
    labpm = np.ascontiguousarray(lab.transpose(1, 2, 0, 3)).astype(
        ml_dtypes.bfloat16)                     # [K, P, B, F]
    labmaps = [labpm[k] for k in range(NCORES)]
    return xmaps, labmaps


def _run(nc, in_maps, out_name):
    import os
    if os.environ.get("K_SIM", "0") == "1":
        import concourse.bass_interp as bass_interp
        sim = bass_interp.MultiCoreSim(nc, NCORES)
        for k in range(NCORES):
            for name, arr in in_maps[k].items():
                sim.cores[k].tensor(name)[:] = arr
        sim.simulate()
        return [{out_name: sim.cores[k].tensor(out_name).copy()}
                for k in range(NCORES)]
    from concourse.bass_utils import run_bass_kernel_spmd
    return run_bass_kernel_spmd(
        nc, in_maps, core_ids=list(range(NCORES))).results


def run_a(labmaps):
    nc = _get("a", _build_a)
    import os
    if os.environ.get("K_SIM", "0") == "1":
        import concourse.bass_interp as bass_interp
        sim = bass_interp.MultiCoreSim(nc, NCORES)
        for k in range(NCORES):
            sim.cores[k].tensor("lab")[:] = labmaps[k]
        sim.simulate()
        results = [{"cnt": sim.cores[k].tensor("cnt").copy(),
                    "cnt2": sim.cores[k].tensor("cnt2").copy()}
                   for k in range(NCORES)]
    else:
        from concourse.bass_utils import run_bass_kernel_spmd
        results = run_bass_kernel_spmd(
            nc, [{"lab": lk} for lk in labmaps],
            core_ids=list(range(NCORES))).results
    cnt_g = np.zeros((B, C), dtype=np.float64)
    for r in results:
        # sample 1 (and any Tensor-reduced sample): per-partition values
        cnt_g[1] += r["cnt"].astype(np.float64)[:C, 1]
        # sample 0: ordinary summed columns
        cnt_g[0] += r["cnt2"].astype(np.float64).sum(axis=0)[:C]
    return cnt_g


def run_b(xmaps, labmaps, pattern):
    nc = _get(("b", pattern), _build_b, pattern)
    in_maps = [{"x": xmaps[k], "lab": labmaps[k]} for k in range(NCORES)]
    results = _run(nc, in_maps, "out")
    acc = np.zeros((P, NACC), dtype=np.float64)
    for r in results:
        acc += r["out"].astype(np.float64)
    return acc


def _finish(cnt_g, acc, present, n):
    pad = n.max() - n
    # SEGB/INTB: per-class values live at partition = position in the
    # present-channel list of that sample
    seg = np.zeros((B, C)); inter = np.zeros((B, C))
    for b in range(B):
        pres = np.where(present[b])[0]
        seg[b, pres] = acc[: len(pres), SEGB + b]
        inter[b, pres] = acc[: len(pres), INTB + b]
    cols = acc.sum(axis=0)
    u1 = cols[U1 : U1 + B]
    lse = cols[LSE : LSE + B]
    u2 = np.where(pad > 0, cols[U2 : U2 + B], lse)
    ce = (lse.sum() - (u1.sum() + u2.sum())) / NVOX
    dice_c = 2.0 * inter / (cnt_g + seg + 1e-5)
    dice_i = 1.0 - (present * dice_c).sum(axis=1) / n
    dc = dice_i.mean()
    return np.asarray(0.5 * ce + 0.5 * dc, dtype=np.float32)


def kernel(net_output, target):
    xmaps, labmaps = _shard_inputs(
        np.asarray(net_output), np.asarray(target))
    cnt_g = run_a(labmaps)
    present = cnt_g > 0.5
    n = present.sum(axis=1).astype(np.float64)
    pattern = tuple(tuple(int(c) for c in np.where(present[b])[0])
                    for b in range(B))
    acc = run_b(xmaps, labmaps, pattern)
    return _finish(cnt_g, acc, present, n)


# revision 26
# speedup vs baseline: 1.1100x; 1.0095x over previous
"""DC_CE_Marginal_loss for Trainium2 — 8-core data-parallel Bass kernel.

Shards the [B,C,D,H,W] volume along D across 8 NeuronCores. The one-hot
target is re-encoded host-side as bf16 label planes (exact class indices,
0.41MB/core instead of 6.5MB), and all device tensors are packed
partition-major so each input is a handful of large-descriptor DMAs.

Two launches:

  Launch A (counts): streams the label planes, derives the 16 one-hot
      masks with DVE is_equal compares, and reduces them to per-(b,c)
      voxel counts (sample 0 via ACT accumulators, sample 1 via
      PE matmuls against a ones-column stationary into PSUM). The host
      psums the per-core counts and derives the present-class pattern.

  Launch B (main, compiled per present-pattern): all-bf16 pipeline.
      Per sample: bg-merge adds only the absent channels into channel 0
      (their merge weights are exactly 1), one wide exp over the present
      channels (present bias is 0, so no mask add), pairwise-tree softmax
      denominator, DVE fast reciprocal, then per-channel q = e*r and
      tq = mask*q products that overwrite the dead x/mask planes in SBUF.
      seg_vol / intersect are per-class plane sums done on the idle PE
      (ones-stationary matmuls accumulating in PSUM). The CE dot product
      sum(t*m) is recovered analytically: m_label = ln(q_label) + ln(S)
      with q_label = sum_c tq_c an exact one-hot select, so two ACT
      Ln+accum passes replace the whole t*m multiply/reduce pipeline.

Host sums the per-core partial columns and finishes the loss.
Measured: ~122us HW exec (A ~32us + B ~90us) vs 217us baseline,
rel err ~5e-6.
"""
import numpy as np
import ml_dtypes

B, C, D, H, W = 2, 8, 64, 160, 160
NCORES = 8
DS = D // NCORES            # depth slices per core
PLANE = DS * H * W          # voxels per (b,c) plane per core = 204800
P = 128
FREE = PLANE // P           # 1600
NVOX = B * D * H * W

# launch B accumulator columns.
# SEGB/INTB columns hold per-(b,c) values at PARTITION c (from the matmul
# reduction), so the host reads them per-partition instead of summing.
SEGB = 0           # B cols: seg_vol, value for channel c at partition c
INTB = 2           # B cols: intersect, value for channel c at partition c
U1 = 4             # 2: sum ln(q_label) per sample (summed over partitions)
U2 = 6             # 2: sum ln(S) per sample (only used when pad>0)
LSE = 8            # 2: sum ln(S + pad) per sample
NACC = 10

_CACHE = {}


def _build_a():
    import concourse.bacc as bacc
    import concourse.tile as tile
    from concourse import mybir

    FA = mybir.ActivationFunctionType
    AL = mybir.AluOpType
    f32, bf16 = mybir.dt.float32, mybir.dt.bfloat16

    nc = bacc.Bacc("TRN2", num_devices=NCORES, name="loss_counts_v3")
    # labels per sample, bf16 (values 0..C-1 exact), partition-major
    lab = nc.dram_tensor("lab", [P, B, FREE], bf16, kind="ExternalInput")
    # counts for (b, c) of sample 1 live at partition c of column b;
    # sample-0 counts are per-partition partials in cnt2 columns
    out = nc.dram_tensor("cnt", [P, B], f32, kind="ExternalOutput")
    out2 = nc.dram_tensor("cnt2", [P, B * C], f32, kind="ExternalOutput")

    with tile.TileContext(nc) as tc:
        with (
            tc.tile_pool(name="sb", bufs=1) as sb,
            tc.psum_pool(name="ps", bufs=2) as psp,
        ):
            lab_sb = sb.tile([P, B, FREE], bf16)
            mk = sb.tile([P, B * C, FREE], bf16)
            cnt = sb.tile([P, B], f32)
            nc.vector.memset(cnt[:], 0.0)
            wsl = sb.tile([P, 2 * C - 1], bf16)
            nc.gpsimd.memset(wsl[:], 0.0)
            nc.gpsimd.memset(wsl[:, C - 1 : C], 1.0)
            CH = [(0, 400), (400, 800), (800, 1200), (1200, 1600)]
            nc.sync.dma_start(lab_sb[:], lab[:])
            junk_a = sb.tile([P, FREE], f32)
            # cnt2: per-(b,c) counts as ordinary columns (summed over
            # partitions host-side) for the ACT/DVE-reduced planes
            cnt2 = sb.tile([P, B * C], f32)
            nc.vector.memset(cnt2[:], 0.0)
            # interleave (b0, b1) compares so the ACT (sample 0) and PE
            # (sample 1) reductions both chase the DVE compares plane by
            # plane instead of waiting for a whole sample
            for b in range(B):
                for c in range(C):
                    nc.vector.tensor_scalar(
                        mk[:, b * C + c, :], lab_sb[:, b, :],
                        float(c), None, AL.is_equal)
                if b == 0:
                    for c in range(C):
                        nc.scalar.activation(
                            out=junk_a[:], in_=mk[:, c, :], func=FA.Copy,
                            accum_out=cnt2[:, c : c + 1])
                else:
                    ps = psp.tile([C, 400], f32, tag="ps")
                    items = [(c, j) for c in range(C) for j in range(4)]
                    for idx, (c, j) in enumerate(items):
                        lo, hi = CH[j]
                        nc.tensor.matmul(
                            ps[:, 0 : hi - lo],
                            wsl[:, C - 1 - c : 2 * C - 1 - c],
                            mk[:, b * C + c, lo:hi],
                            start=(idx == 0), stop=(idx == len(items) - 1))
                    nc.vector.tensor_reduce(
                        out=cnt[0:C, b : b + 1], in_=ps[:],
                        axis=mybir.AxisListType.X, op=AL.add)
            nc.sync.dma_start(out[:], cnt[:])
            nc.sync.dma_start(out2[:], cnt2[:])
    nc.compile()
    return nc


def _tree(nc, s4, s2, planes, out_ap=None):
    """Pairwise-add a list of [P, F] APs using slices of the scratch tiles
    s4 ([P,4,F]) / s2 ([P,2,F]) for intermediates. The final add writes
    out_ap if given. Returns the final AP. len(planes) in [2, 8]."""
    from concourse import mybir
    AL = mybir.AluOpType
    scratch = [s4, s2, None]
    cur = list(planes)
    li = 0
    while len(cur) > 1:
        nxt = []
        k = 0
        for i in range(0, len(cur) - 1, 2):
            final = len(cur) == 2
            if final and out_ap is not None:
                dst = out_ap
            elif final and out_ap is None:
                # pick a slot that is never an input at this level
                dst = (s2 if li < 2 else s4)[:, 0, :]
            else:
                dst = scratch[li][:, k, :]
                k += 1
            nc.vector.tensor_tensor(out=dst, in0=cur[i], in1=cur[i + 1],
                                    op=AL.add)
            nxt.append(dst)
        if len(cur) % 2:
            nxt.append(cur[-1])
        cur = nxt
        li += 1
    return cur[0]


def _build_b(pattern):
    """pattern: tuple per sample of present-channel tuples."""
    import concourse.bacc as bacc
    import concourse.tile as tile
    from concourse import mybir

    FA = mybir.ActivationFunctionType
    AL = mybir.AluOpType
    f32, bf16 = mybir.dt.float32, mybir.dt.bfloat16

    pres = [list(p) for p in pattern]
    n = [len(p) for p in pres]
    L = max(n)
    pad = [float(L - nn) for nn in n]
    absent = [[c for c in range(C) if c not in p] for p in pres]

    nc = bacc.Bacc("TRN2", num_devices=NCORES, name="loss_main_v3")
    # partition-major logits + per-sample labels (bf16 class indices)
    x = nc.dram_tensor("x", [P, B * C, FREE], bf16, kind="ExternalInput")
    lab = nc.dram_tensor("lab", [P, B, FREE], bf16, kind="ExternalInput")
    out = nc.dram_tensor("out", [P, NACC], f32, kind="ExternalOutput")

    with tile.TileContext(nc) as tc:
        with (
            tc.tile_pool(name="persist", bufs=1) as persist,
            tc.tile_pool(name="ework", bufs=2) as ework,
            tc.tile_pool(name="small", bufs=2) as small,
            tc.psum_pool(name="ps", bufs=2) as psp,
        ):
            x_sb = persist.tile([P, B * C, FREE], bf16)
            lab_sb = persist.tile([P, B, FREE], bf16)
            # per-(b,c) one-hot mask planes (only present channels),
            # packed per sample; overwritten by tq later
            tslot = {}
            for b in range(B):
                for c in pres[b]:
                    tslot[(b, c)] = len(tslot)
            mk = persist.tile([P, len(tslot), FREE], bf16)
            accs = persist.tile([P, NACC], f32)
            s4 = persist.tile([P, 4, FREE], bf16)
            s2 = persist.tile([P, 2, FREE], bf16)
            junk = persist.tile([P, FREE], bf16)
            psj = persist.tile([P, 400], bf16)
            nc.vector.memset(accs[:], 0.0)
            wsl = persist.tile([P, 2 * C - 1], bf16)
            nc.gpsimd.memset(wsl[:], 0.0)
            nc.gpsimd.memset(wsl[:, C - 1 : C], 1.0)
            CH = [(0, 400), (400, 800), (800, 1200), (1200, 1600)]

            def class_sums(planes, acc_col):
                """acc_col[i] (partition i) = sum over planes[i], via PE."""
                ps = psp.tile([C, 400], f32, tag="ps")
                items = [(c, j) for c in range(len(planes))
                         for j in range(len(CH))]
                for idx, (c, j) in enumerate(items):
                    lo, hi = CH[j]
                    nc.tensor.matmul(
                        ps[:, 0 : hi - lo],
                        wsl[:, C - 1 - c : 2 * C - 1 - c],
                        planes[c][:, lo:hi],
                        start=(idx == 0), stop=(idx == len(items) - 1))
                nc.vector.tensor_reduce(
                    out=acc_col, in_=ps[:],
                    axis=mybir.AxisListType.X, op=AL.add)

            # DMA order: labels first (mask compares start immediately),
            # then per sample absent-x runs, present-x runs
            nc.sync.dma_start(lab_sb[:], lab[:])

            def runs_of(ixs):
                rr = []
                for i in sorted(ixs):
                    if rr and rr[-1][1] == i:
                        rr[-1][1] = i + 1
                    else:
                        rr.append([i, i + 1])
                return rr

            for b in range(B):
                gx = (runs_of([b * C + c for c in absent[b]]) +
                      runs_of([b * C + c for c in pres[b]]))
                for lo, hi in gx:
                    nc.sync.dma_start(x_sb[:, lo:hi, :], x[:, lo:hi, :])

            # one-hot masks from labels (DVE; GpSimd tensor_scalar is
            # ~25x slower than DVE for this)
            for b in range(B):
                for c in pres[b]:
                    nc.vector.tensor_scalar(
                        mk[:, tslot[(b, c)], :], lab_sb[:, b, :],
                        float(c), None, AL.is_equal)

            for b in range(B):
                xb = x_sb[:, b * C : (b + 1) * C, :]
                # ---- bg merge: x0 += sum(absent x) (weights exactly 1)
                if len(absent[b]) == 1:
                    nc.vector.tensor_tensor(
                        out=xb[:, 0, :], in0=xb[:, 0, :],
                        in1=xb[:, absent[b][0], :], op=AL.add)
                elif absent[b]:
                    bg = _tree(nc, s4, s2, [xb[:, c, :] for c in absent[b]])
                    nc.vector.tensor_tensor(
                        out=xb[:, 0, :], in0=xb[:, 0, :], in1=bg, op=AL.add)

                # ---- e = exp(x) over present channels (contiguous runs)
                e = ework.tile([P, C, FREE], bf16, tag="e")
                runs = []
                for c in pres[b]:
                    if runs and runs[-1][1] == c:
                        runs[-1][1] = c + 1
                    else:
                        runs.append([c, c + 1])
                for lo, hi in runs:
                    nc.scalar.activation(
                        out=e[:, lo:hi, :], in_=xb[:, lo:hi, :], func=FA.Exp)

                # ---- S = sum_present e (f32); r = 1/S on DVE (ACT r
                #      would put its Exp<->Ln table flips on the critical
                #      path); Ln(S) accums feed the CE terms
                S = small.tile([P, FREE], f32, tag="S")
                _tree(nc, s4, s2, [e[:, c, :] for c in pres[b]], out_ap=S[:])
                rf = small.tile([P, FREE], f32, tag="rf")
                nc.vector.reciprocal_approx_fast(rf[:], S[:])
                r = small.tile([P, FREE], bf16, tag="r")
                nc.vector.tensor_scalar(r[:], rf[:], 1.0, None, AL.mult)
                if pad[b] > 0:
                    nc.scalar.activation(
                        out=junk[:], in_=S[:], func=FA.Ln,
                        accum_out=accs[:, U2 + b : U2 + b + 1])
                    padb = small.tile([P, 1], f32, tag="pad")
                    nc.vector.memset(padb[:], pad[b])
                    nc.scalar.activation(
                        out=junk[:], in_=S[:], func=FA.Ln, bias=padb[:],
                        accum_out=accs[:, LSE + b : LSE + b + 1])
                else:
                    nc.scalar.activation(
                        out=junk[:], in_=S[:], func=FA.Ln,
                        accum_out=accs[:, LSE + b : LSE + b + 1])

                # ---- q_c = e_c * r, overwriting x planes; seg via PE
                for c in pres[b]:
                    nc.vector.tensor_tensor(
                        out=xb[:, c, :], in0=e[:, c, :], in1=r[:],
                        op=AL.mult)
                class_sums([xb[:, c, :] for c in pres[b]],
                           accs[0:C, SEGB + b : SEGB + b + 1])

                # ---- tq_c = mask_c * q_c, overwriting mask planes;
                #      intersect via PE (GpSimd SBUF traffic poisons DVE
                #      throughput, so everything elementwise stays on DVE)
                for c in pres[b]:
                    sl = tslot[(b, c)]
                    nc.vector.tensor_tensor(
                        out=mk[:, sl, :], in0=mk[:, sl, :],
                        in1=xb[:, c, :], op=AL.mult)
                class_sums([mk[:, tslot[(b, c)], :] for c in pres[b]],
                           accs[0:C, INTB + b : INTB + b + 1])

                # ---- g_q = sum_c tq_c = q_label (exact one-hot select)
                gq = small.tile([P, FREE], bf16, tag="gq")
                _tree(nc, s4, s2,
                      [mk[:, tslot[(b, c)], :] for c in pres[b]],
                      out_ap=gq[:])
                nc.scalar.activation(
                    out=junk[:], in_=gq[:], func=FA.Ln,
                    accum_out=accs[:, U1 + b : U1 + b + 1])

            nc.sync.dma_start(out[:], accs[:])
    nc.compile()
    return nc


def _get(name, builder, *args):
    if name not in _CACHE:
        _CACHE[name] = builder(*args)
    return _CACHE[name]


def _shard_inputs(net_output, target):
    # [B,C,K,P,F] -> per-core partition-major [P, B*C, F] logits plus
    # per-core [P, B, F] label planes (exact bf16 class indices)
    xs = np.asarray(net_output).reshape(B, C, NCORES, P, FREE)
    xpm = np.ascontiguousarray(
        xs.transpose(2, 3, 0, 1, 4).reshape(NCORES, P, B * C, FREE))
    xmaps = [xpm[k].astype(ml_dtypes.bfloat16) for k in range(NCORES)]
    # labels = argmax over one-hot = dot with channel indices (exact)
    ts = np.asarray(target).reshape(B, C, NCORES, P, FREE)
    lab = np.einsum("bckpf,c->bkpf", ts, np.arange(C, dtype=np.float32))
    labpm = np.ascontiguousarray(lab.transpose(1, 2, 0, 3)).astype(
        ml_dtypes.bfloat16)                     # [K, P, B, F]
    labmaps = [labpm[k] for k in range(NCORES)]
    return xmaps, labmaps


def _run(nc, in_maps, out_name):
    import os
    if os.environ.get("K_SIM", "0") == "1":
        import concourse.bass_interp as bass_interp
        sim = bass_interp.MultiCoreSim(nc, NCORES)
        for k in range(NCORES):
            for name, arr in in_maps[k].items():
                sim.cores[k].tensor(name)[:] = arr
        sim.simulate()
        return [{out_name: sim.cores[k].tensor(out_name).copy()}
                for k in range(NCORES)]
    from concourse.bass_utils import run_bass_kernel_spmd
    return run_bass_kernel_spmd(
        nc, in_maps, core_ids=list(range(NCORES))).results


def run_a(labmaps):
    nc = _get("a", _build_a)
    import os
    if os.environ.get("K_SIM", "0") == "1":
        import concourse.bass_interp as bass_interp
        sim = bass_interp.MultiCoreSim(nc, NCORES)
        for k in range(NCORES):
            sim.cores[k].tensor("lab")[:] = labmaps[k]
        sim.simulate()
        results = [{"cnt": sim.cores[k].tensor("cnt").copy(),
                    "cnt2": sim.cores[k].tensor("cnt2").copy()}
                   for k in range(NCORES)]
    else:
        from concourse.bass_utils import run_bass_kernel_spmd
        results = run_bass_kernel_spmd(
            nc, [{"lab": lk} for lk in labmaps],
            core_ids=list(range(NCORES))).results
    cnt_g = np.zeros((B, C), dtype=np.float64)
    for r in results:
        # sample 1 (and any Tensor-reduced sample): per-partition values
        cnt_g[1] += r["cnt"].astype(np.float64)[:C, 1]
        # sample 0: ordinary summed columns
        cnt_g[0] += r["cnt2"].astype(np.float64).sum(axis=0)[:C]
    return cnt_g


def run_b(xmaps, labmaps, pattern):
    nc = _get(("b", pattern), _build_b, pattern)
    in_maps = [{"x": xmaps[k], "lab": labmaps[k]} for k in range(NCORES)]
    results = _run(nc, in_maps, "out")
    acc = np.zeros((P, NACC), dtype=np.float64)
    for r in results:
        acc += r["out"].astype(np.float64)
    return acc


def _finish(cnt_g, acc, present, n):
    pad = n.max() - n
    # SEGB/INTB: per-class values live at partition = position in the
    # present-channel list of that sample
    seg = np.zeros((B, C)); inter = np.zeros((B, C))
    for b in range(B):
        pres = np.where(present[b])[0]
        seg[b, pres] = acc[: len(pres), SEGB + b]
        inter[b, pres] = acc[: len(pres), INTB + b]
    cols = acc.sum(axis=0)
    u1 = cols[U1 : U1 + B]
    lse = cols[LSE : LSE + B]
    u2 = np.where(pad > 0, cols[U2 : U2 + B], lse)
    ce = (lse.sum() - (u1.sum() + u2.sum())) / NVOX
    dice_c = 2.0 * inter / (cnt_g + seg + 1e-5)
    dice_i = 1.0 - (present * dice_c).sum(axis=1) / n
    dc = dice_i.mean()
    return np.asarray(0.5 * ce + 0.5 * dc, dtype=np.float32)


def kernel(net_output, target):
    xmaps, labmaps = _shard_inputs(
        np.asarray(net_output), np.asarray(target))
    cnt_g = run_a(labmaps)
    present = cnt_g > 0.5
    n = present.sum(axis=1).astype(np.float64)
    pattern = tuple(tuple(int(c) for c in np.where(present[b])[0])
                    for b in range(B))
    acc = run_b(xmaps, labmaps, pattern)
    return _finish(cnt_g, acc, present, n)


# revision 28
# speedup vs baseline: 1.1422x; 1.0290x over previous
"""DC_CE_Marginal_loss for Trainium2 — 8-core data-parallel Bass kernel.

Shards the [B,C,D,H,W] volume along D across 8 NeuronCores. The one-hot
target is re-encoded host-side as bf16 label planes (exact class indices,
0.41MB/core instead of 6.5MB), and all device tensors are packed
partition-major so each input is a handful of large-descriptor DMAs.

Two launches:

  Launch A (counts): streams the label planes, derives the 16 one-hot
      masks with DVE is_equal compares, and reduces them to per-(b,c)
      voxel counts (sample 0 via ACT accumulators, sample 1 via
      PE matmuls against a ones-column stationary into PSUM). The host
      psums the per-core counts and derives the present-class pattern.

  Launch B (main, compiled per present-pattern): all-bf16 pipeline.
      Per sample: bg-merge adds only the absent channels into channel 0
      (their merge weights are exactly 1), one wide exp over the present
      channels (present bias is 0, so no mask add), pairwise-tree softmax
      denominator, DVE fast reciprocal, then per-channel q = e*r and
      tq = mask*q products that overwrite the dead x/mask planes in SBUF.
      seg_vol / intersect are per-class plane sums done on the idle PE
      (ones-stationary matmuls accumulating in PSUM; DVE psum tails
      deferred to the queue end). The CE dot product sum(t*m) is
      recovered analytically: m_label = ln(q_label) + ln(S) with
      q_label = sum_c tq_c an exact one-hot select, so two ACT Ln+accum
      passes replace the whole t*m multiply/reduce pipeline. DMA issue
      order feeds the critical path: absent planes -> x0 -> labels ->
      rest of sample 0 -> sample 1.

Host sums the per-core partial columns and finishes the loss.
"""
import numpy as np
import ml_dtypes

B, C, D, H, W = 2, 8, 64, 160, 160
NCORES = 8
DS = D // NCORES            # depth slices per core
PLANE = DS * H * W          # voxels per (b,c) plane per core = 204800
P = 128
FREE = PLANE // P           # 1600
NVOX = B * D * H * W

# launch B accumulator columns.
# SEGB/INTB columns hold per-(b,c) values at PARTITION c (from the matmul
# reduction), so the host reads them per-partition instead of summing.
SEGB = 0           # B cols: seg_vol, value for channel c at partition c
INTB = 2           # B cols: intersect, value for channel c at partition c
U1 = 4             # 2: sum ln(q_label) per sample (summed over partitions)
U2 = 6             # 2: sum ln(S) per sample (only used when pad>0)
LSE = 8            # 2: sum ln(S + pad) per sample
NACC = 10

_CACHE = {}


def _build_a():
    import concourse.bacc as bacc
    import concourse.tile as tile
    from concourse import mybir

    FA = mybir.ActivationFunctionType
    AL = mybir.AluOpType
    f32, bf16 = mybir.dt.float32, mybir.dt.bfloat16

    nc = bacc.Bacc("TRN2", num_devices=NCORES, name="loss_counts_v3")
    # labels per sample, bf16 (values 0..C-1 exact), partition-major
    lab = nc.dram_tensor("lab", [P, B, FREE], bf16, kind="ExternalInput")
    # counts for (b, c) of sample 1 live at partition c of column b;
    # sample-0 counts are per-partition partials in cnt2 columns
    out = nc.dram_tensor("cnt", [P, B], f32, kind="ExternalOutput")
    out2 = nc.dram_tensor("cnt2", [P, B * C], f32, kind="ExternalOutput")

    with tile.TileContext(nc) as tc:
        with (
            tc.tile_pool(name="sb", bufs=1) as sb,
            tc.psum_pool(name="ps", bufs=2) as psp,
        ):
            lab_sb = sb.tile([P, B, FREE], bf16)
            mk = sb.tile([P, B * C, FREE], bf16)
            cnt = sb.tile([P, B], f32)
            nc.vector.memset(cnt[:], 0.0)
            wsl = sb.tile([P, 2 * C - 1], bf16)
            nc.gpsimd.memset(wsl[:], 0.0)
            nc.gpsimd.memset(wsl[:, C - 1 : C], 1.0)
            CH = [(0, 400), (400, 800), (800, 1200), (1200, 1600)]
            nc.sync.dma_start(lab_sb[:], lab[:])
            junk_a = sb.tile([P, FREE], f32)
            # cnt2: per-(b,c) counts as ordinary columns (summed over
            # partitions host-side) for the ACT-reduced planes
            cnt2 = sb.tile([P, B * C], f32)
            nc.vector.memset(cnt2[:], 0.0)
            for b in range(B):
                for c in range(C):
                    nc.vector.tensor_scalar(
                        mk[:, b * C + c, :], lab_sb[:, b, :],
                        float(c), None, AL.is_equal)
                if b == 0:
                    for c in range(C):
                        nc.scalar.activation(
                            out=junk_a[:], in_=mk[:, c, :], func=FA.Copy,
                            accum_out=cnt2[:, c : c + 1])
                else:
                    ps = psp.tile([C, 400], f32, tag="ps")
                    items = [(c, j) for c in range(C) for j in range(4)]
                    for idx, (c, j) in enumerate(items):
                        lo, hi = CH[j]
                        nc.tensor.matmul(
                            ps[:, 0 : hi - lo],
                            wsl[:, C - 1 - c : 2 * C - 1 - c],
                            mk[:, b * C + c, lo:hi],
                            start=(idx == 0), stop=(idx == len(items) - 1))
                    nc.vector.tensor_reduce(
                        out=cnt[0:C, b : b + 1], in_=ps[:],
                        axis=mybir.AxisListType.X, op=AL.add)
            nc.sync.dma_start(out[:], cnt[:])
            nc.sync.dma_start(out2[:], cnt2[:])
    nc.compile()
    return nc


def _tree(nc, s4, s2, planes, out_ap=None):
    """Pairwise-add a list of [P, F] APs using slices of the scratch tiles
    s4 ([P,4,F]) / s2 ([P,2,F]) for intermediates. The final add writes
    out_ap if given. Returns the final AP. len(planes) in [2, 8]."""
    from concourse import mybir
    AL = mybir.AluOpType
    scratch = [s4, s2, None]
    cur = list(planes)
    li = 0
    while len(cur) > 1:
        nxt = []
        k = 0
        for i in range(0, len(cur) - 1, 2):
            final = len(cur) == 2
            if final and out_ap is not None:
                dst = out_ap
            elif final and out_ap is None:
                # pick a slot that is never an input at this level
                dst = (s2 if li < 2 else s4)[:, 0, :]
            else:
                dst = scratch[li][:, k, :]
                k += 1
            nc.vector.tensor_tensor(out=dst, in0=cur[i], in1=cur[i + 1],
                                    op=AL.add)
            nxt.append(dst)
        if len(cur) % 2:
            nxt.append(cur[-1])
        cur = nxt
        li += 1
    return cur[0]


def _build_b(pattern):
    """pattern: tuple per sample of present-channel tuples."""
    import concourse.bacc as bacc
    import concourse.tile as tile
    from concourse import mybir

    FA = mybir.ActivationFunctionType
    AL = mybir.AluOpType
    f32, bf16 = mybir.dt.float32, mybir.dt.bfloat16

    pres = [list(p) for p in pattern]
    n = [len(p) for p in pres]
    L = max(n)
    pad = [float(L - nn) for nn in n]
    absent = [[c for c in range(C) if c not in p] for p in pres]

    nslots = sum(len(p) for p in pres)
    nc = bacc.Bacc("TRN2", num_devices=NCORES, name="loss_main_v3")
    # partition-major logits + per-sample labels (bf16 class indices)
    x = nc.dram_tensor("x", [P, B * C, FREE], bf16, kind="ExternalInput")
    lab = nc.dram_tensor("lab", [P, B, FREE], bf16, kind="ExternalInput")
    out = nc.dram_tensor("out", [P, NACC], f32, kind="ExternalOutput")

    with tile.TileContext(nc) as tc:
        with (
            tc.tile_pool(name="persist", bufs=1) as persist,
            tc.tile_pool(name="ework", bufs=2) as ework,
            tc.tile_pool(name="small", bufs=2) as small,
            tc.psum_pool(name="ps", bufs=4) as psp,
        ):
            x_sb = persist.tile([P, B * C, FREE], bf16)
            lab_sb = persist.tile([P, B, FREE], bf16)
            tslot = {}
            for b in range(B):
                for c in pres[b]:
                    tslot[(b, c)] = len(tslot)
            mk = persist.tile([P, len(tslot), FREE], bf16)
            accs = persist.tile([P, NACC], f32)
            s4 = persist.tile([P, 4, FREE], bf16)
            s2 = persist.tile([P, 2, FREE], bf16)
            junk = persist.tile([P, FREE], bf16)
            nc.vector.memset(accs[:], 0.0)
            wsl = persist.tile([P, 2 * C - 1], bf16)
            nc.gpsimd.memset(wsl[:], 0.0)
            nc.gpsimd.memset(wsl[:, C - 1 : C], 1.0)
            CH = [(0, 400), (400, 800), (800, 1200), (1200, 1600)]

            deferred_tails = []

            def class_sums(planes, acc_col):
                """Per-class plane sums on the PE; the cheap DVE tail
                (psum -> acc col) is deferred to the end of the DVE queue
                so it never head-of-line-blocks the main chain."""
                ps = psp.tile([C, 400], f32, tag="ps")
                items = [(c, j) for c in range(len(planes))
                         for j in range(len(CH))]
                for idx, (c, j) in enumerate(items):
                    lo, hi = CH[j]
                    nc.tensor.matmul(
                        ps[:, 0 : hi - lo],
                        wsl[:, C - 1 - c : 2 * C - 1 - c],
                        planes[c][:, lo:hi],
                        start=(idx == 0), stop=(idx == len(items) - 1))
                deferred_tails.append((ps, acc_col))

            def runs_of(ixs):
                rr = []
                for i in sorted(ixs):
                    if rr and rr[-1][1] == i:
                        rr[-1][1] = i + 1
                    else:
                        rr.append([i, i + 1])
                return rr

            # DMA order tuned for the critical path: sample-0 absent
            # planes (bg tree) -> x0 (merge target) -> labels (masks) ->
            # rest of sample-0 present -> sample 1
            g0 = runs_of([0 * C + c for c in absent[0]])
            p0 = runs_of([0 * C + c for c in pres[0]])
            first_groups = list(g0)
            rest0 = []
            if p0:
                lo, hi = p0[0]
                first_groups.append([lo, lo + 1])
                if hi > lo + 1:
                    rest0 = [[lo + 1, hi]] + p0[1:]
                else:
                    rest0 = list(p0[1:])
            for lo, hi in first_groups:
                nc.sync.dma_start(x_sb[:, lo:hi, :], x[:, lo:hi, :])
            nc.sync.dma_start(lab_sb[:], lab[:])
            for lo, hi in rest0:
                nc.sync.dma_start(x_sb[:, lo:hi, :], x[:, lo:hi, :])
            for b in range(1, B):
                gx = (runs_of([b * C + c for c in absent[b]]) +
                      runs_of([b * C + c for c in pres[b]]))
                for lo, hi in gx:
                    nc.sync.dma_start(x_sb[:, lo:hi, :], x[:, lo:hi, :])

            def bg_merge(b):
                xb = x_sb[:, b * C : (b + 1) * C, :]
                if len(absent[b]) == 1:
                    nc.vector.tensor_tensor(
                        out=xb[:, 0, :], in0=xb[:, 0, :],
                        in1=xb[:, absent[b][0], :], op=AL.add)
                elif absent[b]:
                    bg = _tree(nc, s4, s2, [xb[:, c, :] for c in absent[b]])
                    nc.vector.tensor_tensor(
                        out=xb[:, 0, :], in0=xb[:, 0, :], in1=bg, op=AL.add)

            # bg merge for sample 0 first (its inputs land first), then
            # all the one-hot masks — they fill the DVE while exp_b0
            # runs on ACT
            bg_merge(0)
            for b in range(B):
                for c in pres[b]:
                    nc.vector.tensor_scalar(
                        mk[:, tslot[(b, c)], :], lab_sb[:, b, :],
                        float(c), None, AL.is_equal)

            for b in range(B):
                xb = x_sb[:, b * C : (b + 1) * C, :]
                if b > 0:
                    bg_merge(b)

                # ---- e = exp(x) over present channels (contiguous runs)
                e = ework.tile([P, C, FREE], bf16, tag="e")
                runs = []
                for c in pres[b]:
                    if runs and runs[-1][1] == c:
                        runs[-1][1] = c + 1
                    else:
                        runs.append([c, c + 1])
                for lo, hi in runs:
                    nc.scalar.activation(
                        out=e[:, lo:hi, :], in_=xb[:, lo:hi, :], func=FA.Exp)

                # ---- S = sum_present e (f32); r = 1/S on DVE (ACT r
                #      would put Exp<->Ln table flips on the critical path)
                S = small.tile([P, FREE], f32, tag="S")
                _tree(nc, s4, s2, [e[:, c, :] for c in pres[b]], out_ap=S[:])
                rf = small.tile([P, FREE], f32, tag="rf")
                nc.vector.reciprocal_approx_fast(rf[:], S[:])
                r = small.tile([P, FREE], bf16, tag="r")
                nc.vector.tensor_scalar(r[:], rf[:], 1.0, None, AL.mult)
                if pad[b] > 0:
                    nc.scalar.activation(
                        out=junk[:], in_=S[:], func=FA.Ln,
                        accum_out=accs[:, U2 + b : U2 + b + 1])
                    padb = small.tile([P, 1], f32, tag="pad")
                    nc.vector.memset(padb[:], pad[b])
                    nc.scalar.activation(
                        out=junk[:], in_=S[:], func=FA.Ln, bias=padb[:],
                        accum_out=accs[:, LSE + b : LSE + b + 1])
                else:
                    nc.scalar.activation(
                        out=junk[:], in_=S[:], func=FA.Ln,
                        accum_out=accs[:, LSE + b : LSE + b + 1])

                # ---- q_c = e_c * r (overwrites x planes); seg on PE
                for c in pres[b]:
                    nc.vector.tensor_tensor(
                        out=xb[:, c, :], in0=e[:, c, :], in1=r[:],
                        op=AL.mult)
                class_sums([xb[:, c, :] for c in pres[b]],
                           accs[0:C, SEGB + b : SEGB + b + 1])

                # ---- tq_c = mask_c * q_c (overwrites masks); int on PE
                for c in pres[b]:
                    sl = tslot[(b, c)]
                    nc.vector.tensor_tensor(
                        out=mk[:, sl, :], in0=mk[:, sl, :],
                        in1=xb[:, c, :], op=AL.mult)
                class_sums([mk[:, tslot[(b, c)], :] for c in pres[b]],
                           accs[0:C, INTB + b : INTB + b + 1])

                # ---- g_q = sum_c tq_c = q_label (exact one-hot select)
                gq = small.tile([P, FREE], bf16, tag="gq")
                _tree(nc, s4, s2,
                      [mk[:, tslot[(b, c)], :] for c in pres[b]],
                      out_ap=gq[:])
                nc.scalar.activation(
                    out=junk[:], in_=gq[:], func=FA.Ln,
                    accum_out=accs[:, U1 + b : U1 + b + 1])

            # ---- deferred psum tails, then output
            for ps, acc_col in deferred_tails:
                nc.vector.tensor_reduce(
                    out=acc_col, in_=ps[:],
                    axis=mybir.AxisListType.X, op=AL.add)
            nc.sync.dma_start(out[:], accs[:])
    nc.compile()
    return nc


def _get(name, builder, *args):
    if name not in _CACHE:
        _CACHE[name] = builder(*args)
    return _CACHE[name]


def _shard_inputs(net_output, target):
    # [B,C,K,P,F] -> per-core partition-major [P, B*C, F] logits plus
    # per-core [P, B, F] label planes (exact bf16 class indices)
    xs = np.asarray(net_output).reshape(B, C, NCORES, P, FREE)
    xpm = np.ascontiguousarray(
        xs.transpose(2, 3, 0, 1, 4).reshape(NCORES, P, B * C, FREE))
    xmaps = [xpm[k].astype(ml_dtypes.bfloat16) for k in range(NCORES)]
    # labels = argmax over one-hot = dot with channel indices (exact)
    ts = np.asarray(target).reshape(B, C, NCORES, P, FREE)
    lab = np.einsum("bckpf,c->bkpf", ts, np.arange(C, dtype=np.float32))
    labpm = np.ascontiguousarray(lab.transpose(1, 2, 0, 3)).astype(
        ml_dtypes.bfloat16)                     # [K, P, B, F]
    labmaps = [labpm[k] for k in range(NCORES)]
    return xmaps, labmaps


def _run(nc, in_maps, out_name):
    import os
    if os.environ.get("K_SIM", "0") == "1":
        import concourse.bass_interp as bass_interp
        sim = bass_interp.MultiCoreSim(nc, NCORES)
        for k in range(NCORES):
            for name, arr in in_maps[k].items():
                sim.cores[k].tensor(name)[:] = arr
        sim.simulate()
        return [{out_name: sim.cores[k].tensor(out_name).copy()}
                for k in range(NCORES)]
    from concourse.bass_utils import run_bass_kernel_spmd
    return run_bass_kernel_spmd(
        nc, in_maps, core_ids=list(range(NCORES))).results


def run_a(labmaps):
    nc = _get("a", _build_a)
    import os
    if os.environ.get("K_SIM", "0") == "1":
        import concourse.bass_interp as bass_interp
        sim = bass_interp.MultiCoreSim(nc, NCORES)
        for k in range(NCORES):
            sim.cores[k].tensor("lab")[:] = labmaps[k]
        sim.simulate()
        results = [{"cnt": sim.cores[k].tensor("cnt").copy(),
                    "cnt2": sim.cores[k].tensor("cnt2").copy()}
                   for k in range(NCORES)]
    else:
        from concourse.bass_utils import run_bass_kernel_spmd
        results = run_bass_kernel_spmd(
            nc, [{"lab": lk} for lk in labmaps],
            core_ids=list(range(NCORES))).results
    cnt_g = np.zeros((B, C), dtype=np.float64)
    for r in results:
        # sample 1: per-partition values from the PE reduction
        cnt_g[1] += r["cnt"].astype(np.float64)[:C, 1]
        # sample 0: ordinary summed columns
        cnt_g[0] += r["cnt2"].astype(np.float64).sum(axis=0)[:C]
    return cnt_g


def run_b(xmaps, labmaps, pattern):
    nc = _get(("b", pattern), _build_b, pattern)
    in_maps = [{"x": xmaps[k], "lab": labmaps[k]} for k in range(NCORES)]
    results = _run(nc, in_maps, "out")
    acc = np.zeros((P, NACC), dtype=np.float64)
    for r in results:
        acc += r["out"].astype(np.float64)
    return acc


def _finish(cnt_g, acc, present, n):
    pad = n.max() - n
    # SEGB/INTB: per-class values live at partition = position in the
    # present-channel list of that sample
    seg = np.zeros((B, C)); inter = np.zeros((B, C))
    for b in range(B):
        pres = np.where(present[b])[0]
        seg[b, pres] = acc[: len(pres), SEGB + b]
        inter[b, pres] = acc[: len(pres), INTB + b]
    cols = acc.sum(axis=0)
    u1 = cols[U1 : U1 + B]
    lse = cols[LSE : LSE + B]
    u2 = np.where(pad > 0, cols[U2 : U2 + B], lse)
    ce = (lse.sum() - (u1.sum() + u2.sum())) / NVOX
    dice_c = 2.0 * inter / (cnt_g + seg + 1e-5)
    dice_i = 1.0 - (present * dice_c).sum(axis=1) / n
    dc = dice_i.mean()
    return np.asarray(0.5 * ce + 0.5 * dc, dtype=np.float32)


def kernel(net_output, target):
    xmaps, labmaps = _shard_inputs(
        np.asarray(net_output), np.asarray(target))
    cnt_g = run_a(labmaps)
    present = cnt_g > 0.5
    n = present.sum(axis=1).astype(np.float64)
    pattern = tuple(tuple(int(c) for c in np.where(present[b])[0])
                    for b in range(B))
    acc = run_b(xmaps, labmaps, pattern)
    return _finish(cnt_g, acc, present, n)


# revision 29
# speedup vs baseline: 1.1497x; 1.0066x over previous
"""DC_CE_Marginal_loss for Trainium2 — 8-core data-parallel Bass kernel.

Shards the [B,C,D,H,W] volume along D across 8 NeuronCores. The one-hot
target is re-encoded host-side as bf16 label planes (exact class indices,
0.41MB/core instead of 6.5MB), and all device tensors are packed
partition-major so each input is a handful of large-descriptor DMAs.

Two launches:

  Launch A (counts): streams the label planes, derives the 16 one-hot
      masks with DVE is_equal compares, and reduces them to per-(b,c)
      voxel counts (sample 0 via ACT accumulators, sample 1 via
      PE matmuls against a ones-column stationary into PSUM). The host
      psums the per-core counts and derives the present-class pattern.

  Launch B (main, compiled per present-pattern): all-bf16 pipeline.
      Per sample: bg-merge adds only the absent channels into channel 0
      (their merge weights are exactly 1), one wide exp over the present
      channels (present bias is 0, so no mask add), pairwise-tree softmax
      denominator, DVE fast reciprocal, then per-channel q = e*r and
      tq = mask*q products that overwrite the dead x/mask planes in SBUF.
      seg_vol / intersect are per-class plane sums done on the idle PE
      (ones-stationary matmuls accumulating in PSUM; DVE psum tails
      deferred to the queue end). The CE dot product sum(t*m) is
      recovered analytically: m_label = ln(q_label) + ln(S) with
      q_label = sum_c tq_c an exact one-hot select, so two ACT Ln+accum
      passes replace the whole t*m multiply/reduce pipeline. DMA issue
      order feeds the critical path: absent planes -> x0 -> labels ->
      rest of sample 0 -> sample 1.

Host sums the per-core partial columns and finishes the loss.
"""
import numpy as np
import ml_dtypes

B, C, D, H, W = 2, 8, 64, 160, 160
NCORES = 8
DS = D // NCORES            # depth slices per core
PLANE = DS * H * W          # voxels per (b,c) plane per core = 204800
P = 128
FREE = PLANE // P           # 1600
NVOX = B * D * H * W

# launch B accumulator columns.
# SEGB/INTB columns hold per-(b,c) values at PARTITION c (from the matmul
# reduction), so the host reads them per-partition instead of summing.
SEGB = 0           # B cols: seg_vol, value for channel c at partition c
INTB = 2           # B cols: intersect, value for channel c at partition c
U1 = 4             # 2: sum ln(q_label) per sample (summed over partitions)
U2 = 6             # 2: sum ln(S) per sample (only used when pad>0)
LSE = 8            # 2: sum ln(S + pad) per sample
NACC = 10

_CACHE = {}


def _build_a():
    import concourse.bacc as bacc
    import concourse.tile as tile
    from concourse import mybir

    FA = mybir.ActivationFunctionType
    AL = mybir.AluOpType
    f32, bf16 = mybir.dt.float32, mybir.dt.bfloat16

    nc = bacc.Bacc("TRN2", num_devices=NCORES, name="loss_counts_v3")
    # labels per sample, bf16 (values 0..C-1 exact), partition-major
    lab = nc.dram_tensor("lab", [P, B, FREE], bf16, kind="ExternalInput")
    # counts for (b, c) of sample 1 live at partition c of column b;
    # sample-0 counts are per-partition partials in cnt2 columns
    out = nc.dram_tensor("cnt", [P, B], f32, kind="ExternalOutput")
    out2 = nc.dram_tensor("cnt2", [P, B * C], f32, kind="ExternalOutput")

    with tile.TileContext(nc) as tc:
        with (
            tc.tile_pool(name="sb", bufs=1) as sb,
            tc.psum_pool(name="ps", bufs=2) as psp,
        ):
            lab_sb = sb.tile([P, B, FREE], bf16)
            mk = sb.tile([P, B * C, FREE], bf16)
            cnt = sb.tile([P, B], f32)
            nc.vector.memset(cnt[:], 0.0)
            wsl = sb.tile([P, 2 * C - 1], bf16)
            nc.gpsimd.memset(wsl[:], 0.0)
            nc.gpsimd.memset(wsl[:, C - 1 : C], 1.0)
            CH = [(0, 400), (400, 800), (800, 1200), (1200, 1600)]
            nc.sync.dma_start(lab_sb[:], lab[:])
            junk_a = sb.tile([P, FREE], f32)
            # cnt2: per-(b,c) counts as ordinary columns (summed over
            # partitions host-side) for the ACT-reduced planes
            cnt2 = sb.tile([P, B * C], f32)
            nc.vector.memset(cnt2[:], 0.0)
            for b in range(B):
                for c in range(C):
                    nc.vector.tensor_scalar(
                        mk[:, b * C + c, :], lab_sb[:, b, :],
                        float(c), None, AL.is_equal)
                if b == 0:
                    for c in range(C):
                        nc.scalar.activation(
                            out=junk_a[:], in_=mk[:, c, :], func=FA.Copy,
                            accum_out=cnt2[:, c : c + 1])
                else:
                    ps = psp.tile([C, 400], f32, tag="ps")
                    items = [(c, j) for c in range(C) for j in range(4)]
                    for idx, (c, j) in enumerate(items):
                        lo, hi = CH[j]
                        nc.tensor.matmul(
                            ps[:, 0 : hi - lo],
                            wsl[:, C - 1 - c : 2 * C - 1 - c],
                            mk[:, b * C + c, lo:hi],
                            start=(idx == 0), stop=(idx == len(items) - 1))
                    nc.vector.tensor_reduce(
                        out=cnt[0:C, b : b + 1], in_=ps[:],
                        axis=mybir.AxisListType.X, op=AL.add)
            nc.sync.dma_start(out[:], cnt[:])
            nc.sync.dma_start(out2[:], cnt2[:])
    nc.compile()
    return nc


def _tree(nc, s4, s2, planes, out_ap=None):
    """Pairwise-add a list of [P, F] APs using slices of the scratch tiles
    s4 ([P,4,F]) / s2 ([P,2,F]) for intermediates. The final add writes
    out_ap if given. Returns the final AP. len(planes) in [2, 8]."""
    from concourse import mybir
    AL = mybir.AluOpType
    scratch = [s4, s2, None]
    cur = list(planes)
    li = 0
    while len(cur) > 1:
        nxt = []
        k = 0
        for i in range(0, len(cur) - 1, 2):
            final = len(cur) == 2
            if final and out_ap is not None:
                dst = out_ap
            elif final and out_ap is None:
                # pick a slot that is never an input at this level
                dst = (s2 if li < 2 else s4)[:, 0, :]
            else:
                dst = scratch[li][:, k, :]
                k += 1
            nc.vector.tensor_tensor(out=dst, in0=cur[i], in1=cur[i + 1],
                                    op=AL.add)
            nxt.append(dst)
        if len(cur) % 2:
            nxt.append(cur[-1])
        cur = nxt
        li += 1
    return cur[0]


def _build_b(pattern):
    """pattern: tuple per sample of present-channel tuples."""
    import concourse.bacc as bacc
    import concourse.tile as tile
    from concourse import mybir

    FA = mybir.ActivationFunctionType
    AL = mybir.AluOpType
    f32, bf16 = mybir.dt.float32, mybir.dt.bfloat16

    pres = [list(p) for p in pattern]
    n = [len(p) for p in pres]
    L = max(n)
    pad = [float(L - nn) for nn in n]
    absent = [[c for c in range(C) if c not in p] for p in pres]

    nslots = sum(len(p) for p in pres)
    nc = bacc.Bacc("TRN2", num_devices=NCORES, name="loss_main_v3")
    # partition-major logits + per-sample labels (bf16 class indices)
    x = nc.dram_tensor("x", [P, B * C, FREE], bf16, kind="ExternalInput")
    lab = nc.dram_tensor("lab", [P, B, FREE], bf16, kind="ExternalInput")
    out = nc.dram_tensor("out", [P, NACC], f32, kind="ExternalOutput")

    with tile.TileContext(nc) as tc:
        with (
            tc.tile_pool(name="persist", bufs=1) as persist,
            tc.tile_pool(name="ework", bufs=2) as ework,
            tc.tile_pool(name="small", bufs=2) as small,
            tc.psum_pool(name="ps", bufs=4) as psp,
        ):
            x_sb = persist.tile([P, B * C, FREE], bf16)
            lab_sb = persist.tile([P, B, FREE], bf16)
            tslot = {}
            for b in range(B):
                for c in pres[b]:
                    tslot[(b, c)] = len(tslot)
            mk = persist.tile([P, len(tslot), FREE], bf16)
            accs = persist.tile([P, NACC], f32)
            s4 = persist.tile([P, 4, FREE], bf16)
            s2 = persist.tile([P, 2, FREE], bf16)
            junk = persist.tile([P, FREE], bf16)
            nc.vector.memset(accs[:], 0.0)
            wsl = persist.tile([P, 2 * C - 1], bf16)
            nc.gpsimd.memset(wsl[:], 0.0)
            nc.gpsimd.memset(wsl[:, C - 1 : C], 1.0)
            CH = [(0, 400), (400, 800), (800, 1200), (1200, 1600)]

            deferred_tails = []

            def class_sums(planes, acc_col):
                """Per-class plane sums on the PE; the cheap DVE tail
                (psum -> acc col) is deferred to the end of the DVE queue
                so it never head-of-line-blocks the main chain."""
                ps = psp.tile([C, 400], f32, tag="ps")
                items = [(c, j) for c in range(len(planes))
                         for j in range(len(CH))]
                for idx, (c, j) in enumerate(items):
                    lo, hi = CH[j]
                    nc.tensor.matmul(
                        ps[:, 0 : hi - lo],
                        wsl[:, C - 1 - c : 2 * C - 1 - c],
                        planes[c][:, lo:hi],
                        start=(idx == 0), stop=(idx == len(items) - 1))
                deferred_tails.append((ps, acc_col))

            def runs_of(ixs):
                rr = []
                for i in sorted(ixs):
                    if rr and rr[-1][1] == i:
                        rr[-1][1] = i + 1
                    else:
                        rr.append([i, i + 1])
                return rr

            # DMA order tuned for the critical path: sample-0 absent
            # planes (bg tree) -> x0 (merge target) -> labels (masks) ->
            # rest of sample-0 present -> sample 1
            g0 = runs_of([0 * C + c for c in absent[0]])
            p0 = runs_of([0 * C + c for c in pres[0]])
            first_groups = list(g0)
            rest0 = []
            exp0_split = None
            if p0:
                lo, hi = p0[0]
                first_groups.append([lo, lo + 1])
                if hi - lo >= 4:
                    # split the big present run so exp_b0 (and the S tree
                    # behind it) can start on the first half early
                    mid = lo + (hi - lo + 1) // 2 + 1
                    rest0 = [[lo + 1, mid], [mid, hi]] + p0[1:]
                    exp0_split = mid
                elif hi > lo + 1:
                    rest0 = [[lo + 1, hi]] + p0[1:]
                else:
                    rest0 = list(p0[1:])
            for lo, hi in first_groups:
                nc.sync.dma_start(x_sb[:, lo:hi, :], x[:, lo:hi, :])
            nc.sync.dma_start(lab_sb[:], lab[:])
            for lo, hi in rest0:
                nc.sync.dma_start(x_sb[:, lo:hi, :], x[:, lo:hi, :])
            for b in range(1, B):
                gx = (runs_of([b * C + c for c in absent[b]]) +
                      runs_of([b * C + c for c in pres[b]]))
                for lo, hi in gx:
                    nc.sync.dma_start(x_sb[:, lo:hi, :], x[:, lo:hi, :])

            def bg_merge(b):
                xb = x_sb[:, b * C : (b + 1) * C, :]
                if len(absent[b]) == 1:
                    nc.vector.tensor_tensor(
                        out=xb[:, 0, :], in0=xb[:, 0, :],
                        in1=xb[:, absent[b][0], :], op=AL.add)
                elif absent[b]:
                    bg = _tree(nc, s4, s2, [xb[:, c, :] for c in absent[b]])
                    nc.vector.tensor_tensor(
                        out=xb[:, 0, :], in0=xb[:, 0, :], in1=bg, op=AL.add)

            # bg merge for sample 0 first (its inputs land first), then
            # all the one-hot masks — they fill the DVE while exp_b0
            # runs on ACT
            bg_merge(0)
            for b in range(B):
                for c in pres[b]:
                    nc.vector.tensor_scalar(
                        mk[:, tslot[(b, c)], :], lab_sb[:, b, :],
                        float(c), None, AL.is_equal)

            for b in range(B):
                xb = x_sb[:, b * C : (b + 1) * C, :]
                if b > 0:
                    bg_merge(b)

                # ---- e = exp(x) over present channels (contiguous runs)
                e = ework.tile([P, C, FREE], bf16, tag="e")
                runs = []
                for c in pres[b]:
                    if runs and runs[-1][1] == c:
                        runs[-1][1] = c + 1
                    else:
                        runs.append([c, c + 1])
                if b == 0 and exp0_split is not None:
                    lo, hi = runs[0]
                    runs = [[lo, exp0_split], [exp0_split, hi]] + runs[1:]
                for lo, hi in runs:
                    nc.scalar.activation(
                        out=e[:, lo:hi, :], in_=xb[:, lo:hi, :], func=FA.Exp)

                # ---- S = sum_present e (f32); r = 1/S on DVE (ACT r
                #      would put Exp<->Ln table flips on the critical path)
                S = small.tile([P, FREE], f32, tag="S")
                _tree(nc, s4, s2, [e[:, c, :] for c in pres[b]], out_ap=S[:])
                rf = small.tile([P, FREE], f32, tag="rf")
                nc.vector.reciprocal_approx_fast(rf[:], S[:])
                r = small.tile([P, FREE], bf16, tag="r")
                nc.vector.tensor_scalar(r[:], rf[:], 1.0, None, AL.mult)
                if pad[b] > 0:
                    nc.scalar.activation(
                        out=junk[:], in_=S[:], func=FA.Ln,
                        accum_out=accs[:, U2 + b : U2 + b + 1])
                    padb = small.tile([P, 1], f32, tag="pad")
                    nc.vector.memset(padb[:], pad[b])
                    nc.scalar.activation(
                        out=junk[:], in_=S[:], func=FA.Ln, bias=padb[:],
                        accum_out=accs[:, LSE + b : LSE + b + 1])
                else:
                    nc.scalar.activation(
                        out=junk[:], in_=S[:], func=FA.Ln,
                        accum_out=accs[:, LSE + b : LSE + b + 1])

                # ---- q_c = e_c * r (overwrites x planes); seg on PE
                for c in pres[b]:
                    nc.vector.tensor_tensor(
                        out=xb[:, c, :], in0=e[:, c, :], in1=r[:],
                        op=AL.mult)
                class_sums([xb[:, c, :] for c in pres[b]],
                           accs[0:C, SEGB + b : SEGB + b + 1])

                # ---- tq_c = mask_c * q_c (overwrites masks); int on PE
                for c in pres[b]:
                    sl = tslot[(b, c)]
                    nc.vector.tensor_tensor(
                        out=mk[:, sl, :], in0=mk[:, sl, :],
                        in1=xb[:, c, :], op=AL.mult)
                class_sums([mk[:, tslot[(b, c)], :] for c in pres[b]],
                           accs[0:C, INTB + b : INTB + b + 1])

                # ---- g_q = sum_c tq_c = q_label (exact one-hot select)
                gq = small.tile([P, FREE], bf16, tag="gq")
                _tree(nc, s4, s2,
                      [mk[:, tslot[(b, c)], :] for c in pres[b]],
                      out_ap=gq[:])
                nc.scalar.activation(
                    out=junk[:], in_=gq[:], func=FA.Ln,
                    accum_out=accs[:, U1 + b : U1 + b + 1])

            # ---- deferred psum tails, then output
            for ps, acc_col in deferred_tails:
                nc.vector.tensor_reduce(
                    out=acc_col, in_=ps[:],
                    axis=mybir.AxisListType.X, op=AL.add)
            nc.sync.dma_start(out[:], accs[:])
    nc.compile()
    return nc


def _get(name, builder, *args):
    if name not in _CACHE:
        _CACHE[name] = builder(*args)
    return _CACHE[name]


def _shard_inputs(net_output, target):
    # [B,C,K,P,F] -> per-core partition-major [P, B*C, F] logits plus
    # per-core [P, B, F] label planes (exact bf16 class indices)
    xs = np.asarray(net_output).reshape(B, C, NCORES, P, FREE)
    xpm = np.ascontiguousarray(
        xs.transpose(2, 3, 0, 1, 4).reshape(NCORES, P, B * C, FREE))
    xmaps = [xpm[k].astype(ml_dtypes.bfloat16) for k in range(NCORES)]
    # labels = argmax over one-hot = dot with channel indices (exact)
    ts = np.asarray(target).reshape(B, C, NCORES, P, FREE)
    lab = np.einsum("bckpf,c->bkpf", ts, np.arange(C, dtype=np.float32))
    labpm = np.ascontiguousarray(lab.transpose(1, 2, 0, 3)).astype(
        ml_dtypes.bfloat16)                     # [K, P, B, F]
    labmaps = [labpm[k] for k in range(NCORES)]
    return xmaps, labmaps


def _run(nc, in_maps, out_name):
    import os
    if os.environ.get("K_SIM", "0") == "1":
        import concourse.bass_interp as bass_interp
        sim = bass_interp.MultiCoreSim(nc, NCORES)
        for k in range(NCORES):
            for name, arr in in_maps[k].items():
                sim.cores[k].tensor(name)[:] = arr
        sim.simulate()
        return [{out_name: sim.cores[k].tensor(out_name).copy()}
                for k in range(NCORES)]
    from concourse.bass_utils import run_bass_kernel_spmd
    return run_bass_kernel_spmd(
        nc, in_maps, core_ids=list(range(NCORES))).results


def run_a(labmaps):
    nc = _get("a", _build_a)
    import os
    if os.environ.get("K_SIM", "0") == "1":
        import concourse.bass_interp as bass_interp
        sim = bass_interp.MultiCoreSim(nc, NCORES)
        for k in range(NCORES):
            sim.cores[k].tensor("lab")[:] = labmaps[k]
        sim.simulate()
        results = [{"cnt": sim.cores[k].tensor("cnt").copy(),
                    "cnt2": sim.cores[k].tensor("cnt2").copy()}
                   for k in range(NCORES)]
    else:
        from concourse.bass_utils import run_bass_kernel_spmd
        results = run_bass_kernel_spmd(
            nc, [{"lab": lk} for lk in labmaps],
            core_ids=list(range(NCORES))).results
    cnt_g = np.zeros((B, C), dtype=np.float64)
    for r in results:
        # sample 1: per-partition values from the PE reduction
        cnt_g[1] += r["cnt"].astype(np.float64)[:C, 1]
        # sample 0: ordinary summed columns
        cnt_g[0] += r["cnt2"].astype(np.float64).sum(axis=0)[:C]
    return cnt_g


def run_b(xmaps, labmaps, pattern):
    nc = _get(("b", pattern), _build_b, pattern)
    in_maps = [{"x": xmaps[k], "lab": labmaps[k]} for k in range(NCORES)]
    results = _run(nc, in_maps, "out")
    acc = np.zeros((P, NACC), dtype=np.float64)
    for r in results:
        acc += r["out"].astype(np.float64)
    return acc


def _finish(cnt_g, acc, present, n):
    pad = n.max() - n
    # SEGB/INTB: per-class values live at partition = position in the
    # present-channel list of that sample
    seg = np.zeros((B, C)); inter = np.zeros((B, C))
    for b in range(B):
        pres = np.where(present[b])[0]
        seg[b, pres] = acc[: len(pres), SEGB + b]
        inter[b, pres] = acc[: len(pres), INTB + b]
    cols = acc.sum(axis=0)
    u1 = cols[U1 : U1 + B]
    lse = cols[LSE : LSE + B]
    u2 = np.where(pad > 0, cols[U2 : U2 + B], lse)
    ce = (lse.sum() - (u1.sum() + u2.sum())) / NVOX
    dice_c = 2.0 * inter / (cnt_g + seg + 1e-5)
    dice_i = 1.0 - (present * dice_c).sum(axis=1) / n
    dc = dice_i.mean()
    return np.asarray(0.5 * ce + 0.5 * dc, dtype=np.float32)


def kernel(net_output, target):
    xmaps, labmaps = _shard_inputs(
        np.asarray(net_output), np.asarray(target))
    cnt_g = run_a(labmaps)
    present = cnt_g > 0.5
    n = present.sum(axis=1).astype(np.float64)
    pattern = tuple(tuple(int(c) for c in np.where(present[b])[0])
                    for b in range(B))
    acc = run_b(xmaps, labmaps, pattern)
    return _finish(cnt_g, acc, present, n)
